# revision 12
# baseline (speedup 1.0000x reference)
"""GQA attention forward (B=1, T=2048, DIM=2048, H=16, KV=4, HD=128) on 8 trn2 cores.

Sharding: tensor-parallel over heads. Core c owns q-heads {2c, 2c+1} and kv-head
c//2 (kv work duplicated across the pair of cores sharing it).

v2: bf16 matmul pipeline (PSUM accumulate stays f32), single fused
quarter-pipeline (proj+rope -> attention -> wo per 512-row t-quarter, so the
tensor engine never sees a phase barrier), exp/copies on ACT, reciprocal on
ACT (was 3.4us/tile on DVE), wo-stage PSUM->SBUF copies split ACT/DVE.
Host: pre-transposes x/weights into bf16, sums the 8 partial [T, DIM] outputs.
"""

import sys

if "/opt/trn_rl_repo" not in sys.path:
    sys.path.insert(0, "/opt/trn_rl_repo")

import numpy as np

T = 2048
DIM = 2048
H = 16
KV = 4
HD = 128
NCORES = 8
HPC = H // NCORES            # q heads per core = 2
SCALE = float(HD) ** -0.5
ND = DIM // 128              # dim chunks = 16
NT = T // 128                # t blocks = 16
NQC = T // 512               # t quarters = 4

_CACHE = {}


def _build_nc():
    from contextlib import ExitStack

    from concourse import bacc
    import concourse.mybir as mybir
    import concourse.tile as tile
    from concourse.masks import make_identity

    f32 = mybir.dt.float32
    bf16 = mybir.dt.bfloat16
    Exp = mybir.ActivationFunctionType.Exp

    nc = bacc.Bacc("TRN2", target_bir_lowering=False, debug=False,
                   enable_asserts=False)

    xT = nc.dram_tensor("xT", [DIM, T], bf16, kind="ExternalInput").ap()
    wqT = nc.dram_tensor("wqT", [DIM, HPC * HD], bf16, kind="ExternalInput").ap()
    wkT = nc.dram_tensor("wkT", [DIM, HD], bf16, kind="ExternalInput").ap()
    wvT = nc.dram_tensor("wvT", [DIM, HD], bf16, kind="ExternalInput").ap()
    woT = nc.dram_tensor("woT", [HPC * HD, DIM], bf16, kind="ExternalInput").ap()
    cosT = nc.dram_tensor("cosT", [HD, T], bf16, kind="ExternalInput").ap()
    sinT = nc.dram_tensor("sinT", [HD, T], bf16, kind="ExternalInput").ap()
    out = nc.dram_tensor("out", [T, DIM], bf16, kind="ExternalOutput").ap()

    with tile.TileContext(nc) as tc, ExitStack() as ctx:
        const = ctx.enter_context(tc.tile_pool(name="const", bufs=1))
        wpool = ctx.enter_context(tc.tile_pool(name="wts", bufs=1))
        qkv = ctx.enter_context(tc.tile_pool(name="qkv", bufs=1))

        ident = const.tile([128, 128], bf16)
        make_identity(nc, ident)
        ones_s = const.tile([128, 128], bf16)
        nc.vector.memset(ones_s, 1.0)

        qT_s = qkv.tile([128, HPC * T], bf16)
        kT_s = qkv.tile([128, T], bf16)
        v_s = qkv.tile([128, NT * HD], bf16)   # natural [t%128, hd] per t-block
        aT_s = [qkv.tile([128, T], bf16, name=f"aT{h}") for h in range(HPC)]

        # weight/x loads: k first (first PE work), then the rest in need-order
        wk_s = wpool.tile([128, ND, HD], bf16)
        nc.gpsimd.dma_start(wk_s, wkT.rearrange("(d p) n -> p d n", p=128))
        cos_s = const.tile([128, T], bf16)
        sin_s = const.tile([128, T], bf16)

        xpool = ctx.enter_context(tc.tile_pool(name="xp", bufs=18))

        def load_x_quarter(tq):
            xts = []
            for d in range(ND):
                xt = xpool.tile([128, 512], bf16, tag="xt", name=f"xt{tq}_{d}")
                eng = nc.sync if d % 2 == 0 else nc.scalar
                eng.dma_start(
                    xt, xT[d * 128:(d + 1) * 128, tq * 512:(tq + 1) * 512])
                xts.append(xt)
            return xts

        # DMA issue order = need order: wk, x0, wq, x1, wv, cos/sin, x2,
        # wo, x3 (weights for later stages must not delay early x quarters)
        xq = [load_x_quarter(0)]
        wq_s = wpool.tile([128, ND, HPC * HD], bf16)
        nc.gpsimd.dma_start(wq_s, wqT.rearrange("(d p) n -> p d n", p=128))
        xq.append(load_x_quarter(1))
        wv_s = wpool.tile([128, ND, HD], bf16)
        nc.gpsimd.dma_start(wv_s, wvT.rearrange("(d p) n -> p d n", p=128))
        nc.gpsimd.dma_start(cos_s, cosT)
        nc.gpsimd.dma_start(sin_s, sinT)
        wo_s = wpool.tile([128, HPC, DIM], bf16)

        rp = ctx.enter_context(tc.tile_pool(name="rope", bufs=4))
        pps = ctx.enter_context(tc.tile_pool(name="pps", bufs=2, space="PSUM"))
        sps = ctx.enter_context(tc.tile_pool(name="sps", bufs=3, space="PSUM"))
        otp = ctx.enter_context(tc.tile_pool(name="otp", bufs=1, space="PSUM"))
        wops = ctx.enter_context(tc.tile_pool(name="wops", bufs=2, space="PSUM"))
        ppool = ctx.enter_context(tc.tile_pool(name="pp", bufs=20))
        rcp = ctx.enter_context(tc.tile_pool(name="rcp", bufs=2))
        ostage = ctx.enter_context(tc.tile_pool(name="ost", bufs=6))

        def rope(u, c0, t0):
            us = u[:, c0:c0 + 512]
            rot = rp.tile([128, 512], bf16, tag="rot")
            nc.sync.dma_start(rot[0:64, :], us[64:128, :])
            nc.sync.dma_start(rot[64:128, :], us[0:64, :])
            tmp = rp.tile([128, 512], bf16, tag="rtmp")
            nc.vector.tensor_mul(tmp, us, cos_s[:, t0:t0 + 512])
            nc.vector.tensor_mul(rot, rot, sin_s[:, t0:t0 + 512])
            nc.vector.tensor_add(us, tmp, rot)

        def proj(acc_tag, w_ap, xts, dst, c0):
            acc = pps.tile([128, 512], f32, tag="pps", name=acc_tag)
            for d in range(ND):
                nc.tensor.matmul(acc, w_ap(d), xts[d],
                                 start=(d == 0), stop=(d == ND - 1))
            nc.scalar.copy(dst[:, c0:c0 + 512], acc)

        def quarter_proj(tq, xts):
            t0 = tq * 512
            proj(f"k{tq}", lambda d: wk_s[:, d, :], xts, kT_s, t0)
            rope(kT_s, t0, t0)
            for h in range(HPC):
                proj(f"q{h}_{tq}",
                     lambda d, h=h: wq_s[:, d, h * HD:(h + 1) * HD],
                     xts, qT_s, h * T + t0)
                rope(qT_s, h * T + t0, t0)
            proj(f"v{tq}", lambda d: wv_s[:, d, :], xts, vT_stage, t0)
            for tb in range(tq * 4, tq * 4 + 4):
                vt = pps.tile([128, 128], bf16, tag="pps", name=f"vt{tb}")
                nc.tensor.transpose(
                    vt, vT_stage[:, tb * 128:(tb + 1) * 128], ident)
                nc.scalar.copy(v_s[:, tb * HD:(tb + 1) * HD], vt)

        vT_stage = qkv.tile([128, T], bf16)

        def attention(h, qc):
            qTh = qT_s[:, h * T + qc * 512:h * T + (qc + 1) * 512]
            nkb = 4 * qc + 4
            ptiles = []
            for kb in range(nkb):
                s_ps = sps.tile([128, 512], f32, tag="s", name=f"s{h}_{qc}_{kb}")
                nc.tensor.matmul(
                    s_ps, kT_s[:, kb * 128:(kb + 1) * 128], qTh,
                    start=True, stop=True)
                p_sb = ppool.tile([128, 512], bf16, tag="p",
                                  name=f"p{h}_{qc}_{kb}")
                nc.scalar.activation(p_sb, s_ps, Exp, scale=SCALE)
                if kb >= 4 * qc:
                    nc.gpsimd.affine_select(
                        out=p_sb, in_=p_sb,
                        compare_op=mybir.AluOpType.is_ge,
                        fill=0.0, base=qc * 512 - kb * 128,
                        channel_multiplier=-1, pattern=[[1, 512]])
                ptiles.append(p_sb)
            oT = otp.tile([128, 512], f32, tag="oT", name=f"oT{h}_{qc}")
            for kb in range(nkb):
                nc.tensor.matmul(
                    oT, v_s[:, kb * HD:(kb + 1) * HD], ptiles[kb],
                    start=(kb == 0), stop=(kb == nkb - 1))
            # dn = colsum(sum_kb P_kb): pairwise DVE tree then one ones-matmul
            lvl = list(ptiles)
            ti = 0
            while len(lvl) > 1:
                nxt = []
                for i in range(0, len(lvl) - 1, 2):
                    t = ppool.tile([128, 512], bf16, tag="pt",
                                   name=f"pt{h}_{qc}_{ti}")
                    ti += 1
                    nc.vector.tensor_add(t, lvl[i], lvl[i + 1])
                    nxt.append(t)
                if len(lvl) % 2:
                    nxt.append(lvl[-1])
                lvl = nxt
            dn = sps.tile([128, 512], f32, tag="s", name=f"dn{h}_{qc}")
            nc.tensor.matmul(dn, ones_s, lvl[0], start=True, stop=True)
            rec = rcp.tile([128, 512], f32, tag="rec")
            nc.vector.reciprocal_approx_fast(rec, dn)
            nc.vector.tensor_mul(
                aT_s[h][:, qc * 512:(qc + 1) * 512], oT, rec)

        def wo_block(qc):
            for tb in range(qc * 4, qc * 4 + 4):
                for n4 in range(4):
                    op = wops.tile([128, 512], f32, tag="op")
                    for h in range(HPC):
                        nc.tensor.matmul(
                            op, aT_s[h][:, tb * 128:(tb + 1) * 128],
                            wo_s[:, h, n4 * 512:(n4 + 1) * 512],
                            start=(h == 0), stop=(h == HPC - 1))
                    ob = ostage.tile([128, 512], bf16, tag="ob")
                    nc.vector.tensor_copy(ob, op)
                    eng = nc.sync if n4 % 2 == 0 else nc.scalar
                    eng.dma_start(
                        out[tb * 128:(tb + 1) * 128,
                            n4 * 512:(n4 + 1) * 512], ob)

        for tq in range(NQC):
            if tq + 2 <= 3:
                xq.append(load_x_quarter(tq + 2))
            if tq == 0:
                nc.gpsimd.dma_start(
                    wo_s, woT.rearrange("(h p) n -> p h n", p=128))
            quarter_proj(tq, xq[tq])
            for h in range(HPC):
                attention(h, tq)
            wo_block(tq)

    nc.compile()
    return nc


def _shard_inputs(x, wq, wk, wv, wo, cos, sin):
    import ml_dtypes

    bf = ml_dtypes.bfloat16
    xTh = np.ascontiguousarray(x.reshape(T, DIM).T).astype(bf)
    cosTh = np.ascontiguousarray(cos.T).astype(bf)
    # rotate_half sign fold: out = u*cos + u_rot*sin_signed
    sinTh = np.ascontiguousarray(sin.T).copy()
    sinTh[: HD // 2, :] *= -1.0
    sinTh = sinTh.astype(bf)
    in_maps = []
    for c in range(NCORES):
        g = c // 2
        in_maps.append({
            "xT": xTh,
            "wqT": np.ascontiguousarray(
                wq[c * HPC * HD:(c + 1) * HPC * HD, :].T).astype(bf),
            "wkT": np.ascontiguousarray(wk[g * HD:(g + 1) * HD, :].T).astype(bf),
            "wvT": np.ascontiguousarray(wv[g * HD:(g + 1) * HD, :].T).astype(bf),
            "woT": np.ascontiguousarray(
                wo[:, c * HPC * HD:(c + 1) * HPC * HD].T).astype(bf),
            "cosT": cosTh,
            "sinT": sinTh,
        })
    return in_maps


def _get_exec():
    """Build (once) a cached jitted SPMD executable over the 8 cores.

    Mirrors bass2jax.run_bass_via_pjrt's multi-core branch, but caches the
    jitted callable so repeat kernel() calls don't re-trace/re-lower.
    """
    if "exec" in _CACHE:
        return _CACHE["exec"]

    import jax
    from jax.sharding import Mesh, PartitionSpec
    from jax.experimental.shard_map import shard_map
    from concourse import bass2jax
    import concourse.mybir as mybir

    if "nc" not in _CACHE:
        _CACHE["nc"] = _build_nc()
    nc = _CACHE["nc"]

    bass2jax.install_neuronx_cc_hook()

    part_name = (nc.partition_id_tensor.name
                 if nc.partition_id_tensor else None)
    in_names, out_names, out_avals = [], [], []
    for alloc in nc.m.functions[0].allocations:
        if not isinstance(alloc, mybir.MemoryLocationSet):
            continue
        name = alloc.memorylocations[0].name
        if alloc.kind == "ExternalInput":
            if name != part_name:
                in_names.append(name)
        elif alloc.kind == "ExternalOutput":
            out_names.append(name)
            out_avals.append(jax.core.ShapedArray(
                tuple(alloc.tensor_shape), mybir.dt.np(alloc.dtype)))

    bind_names = in_names + out_names
    if part_name is not None:
        bind_names = bind_names + [part_name]

    def _body(*args):
        operands = list(args)
        if part_name is not None:
            operands.append(bass2jax.partition_id_tensor())
        outs = bass2jax._bass_exec_p.bind(
            *operands,
            out_avals=tuple(out_avals),
            in_names=tuple(bind_names),
            out_names=tuple(out_names),
            lowering_input_output_aliases=(),
            sim_require_finite=True,
            sim_require_nnan=True,
            nc=nc,
        )
        return tuple(outs)

    devices = jax.devices()[:NCORES]
    mesh = Mesh(np.asarray(devices), ("core",))
    n_in = len(in_names)
    n_out = len(out_names)
    sharded = jax.jit(
        shard_map(
            _body, mesh=mesh,
            in_specs=(PartitionSpec("core"),) * (n_in + n_out),
            out_specs=(PartitionSpec("core"),) * n_out,
            check_rep=False,
        ),
        donate_argnums=tuple(range(n_in, n_in + n_out)),
        keep_unused=True,
    )
    _CACHE["body"] = _body
    _CACHE["exec"] = (sharded, in_names, out_names, out_avals, mesh)
    return _CACHE["exec"]


def _concat_inputs(in_maps, in_names):
    return [
        np.concatenate([in_maps[c][name] for c in range(NCORES)], axis=0)
        for name in in_names
    ]


def _zero_outs(out_avals):
    return [
        np.zeros((NCORES * a.shape[0], *a.shape[1:]), a.dtype)
        for a in out_avals
    ]


def kernel(**inputs):
    sharded, in_names, out_names, out_avals, _ = _get_exec()

    in_maps = _shard_inputs(
        np.asarray(inputs["x"], dtype=np.float32),
        np.asarray(inputs["wq"], dtype=np.float32),
        np.asarray(inputs["wk"], dtype=np.float32),
        np.asarray(inputs["wv"], dtype=np.float32),
        np.asarray(inputs["wo"], dtype=np.float32),
        np.asarray(inputs["cos"], dtype=np.float32),
        np.asarray(inputs["sin"], dtype=np.float32),
    )
    concat_in = _concat_inputs(in_maps, in_names)
    out_arrs = sharded(*concat_in, *_zero_outs(out_avals))

    full = np.asarray(out_arrs[out_names.index("out")])
    acc = full.reshape(NCORES, T, DIM).astype(np.float32).sum(axis=0)
    return acc.reshape(1, T, DIM)


# revision 13
# speedup vs baseline: 1.0274x; 1.0274x over previous
"""GQA attention forward (B=1, T=2048, DIM=2048, H=16, KV=4, HD=128) on 8 trn2 cores.

Sharding: tensor-parallel over heads. Core c owns q-heads {2c, 2c+1} and kv-head
c//2 (kv work duplicated across the pair of cores sharing it).

v2: bf16 matmul pipeline (PSUM accumulate stays f32), single fused
quarter-pipeline (proj+rope -> attention -> wo per 512-row t-quarter, so the
tensor engine never sees a phase barrier), exp/copies on ACT, reciprocal on
ACT (was 3.4us/tile on DVE), wo-stage PSUM->SBUF copies split ACT/DVE.
Host: pre-transposes x/weights into bf16, sums the 8 partial [T, DIM] outputs.
"""

import sys

if "/opt/trn_rl_repo" not in sys.path:
    sys.path.insert(0, "/opt/trn_rl_repo")

import numpy as np

T = 2048
DIM = 2048
H = 16
KV = 4
HD = 128
NCORES = 8
HPC = H // NCORES            # q heads per core = 2
SCALE = float(HD) ** -0.5
ND = DIM // 128              # dim chunks = 16
NT = T // 128                # t blocks = 16
NQC = T // 512               # t quarters = 4

_CACHE = {}


def _build_nc():
    from contextlib import ExitStack

    from concourse import bacc
    import concourse.mybir as mybir
    import concourse.tile as tile
    from concourse.masks import make_identity

    f32 = mybir.dt.float32
    bf16 = mybir.dt.bfloat16
    Exp = mybir.ActivationFunctionType.Exp

    nc = bacc.Bacc("TRN2", target_bir_lowering=False, debug=False,
                   enable_asserts=False)

    xT = nc.dram_tensor("xT", [DIM, T], bf16, kind="ExternalInput").ap()
    wqT = nc.dram_tensor("wqT", [DIM, HPC * HD], bf16, kind="ExternalInput").ap()
    wkT = nc.dram_tensor("wkT", [DIM, HD], bf16, kind="ExternalInput").ap()
    wvT = nc.dram_tensor("wvT", [DIM, HD], bf16, kind="ExternalInput").ap()
    woT = nc.dram_tensor("woT", [HPC * HD, DIM], bf16, kind="ExternalInput").ap()
    cosT = nc.dram_tensor("cosT", [HD, T], bf16, kind="ExternalInput").ap()
    sinT = nc.dram_tensor("sinT", [HD, T], bf16, kind="ExternalInput").ap()
    out = nc.dram_tensor("out", [T, DIM], bf16, kind="ExternalOutput").ap()

    with tile.TileContext(nc) as tc, ExitStack() as ctx:
        const = ctx.enter_context(tc.tile_pool(name="const", bufs=1))
        wpool = ctx.enter_context(tc.tile_pool(name="wts", bufs=1))
        qkv = ctx.enter_context(tc.tile_pool(name="qkv", bufs=1))

        ident = const.tile([128, 128], bf16)
        make_identity(nc, ident)
        ones_s = const.tile([128, 128], bf16)
        nc.vector.memset(ones_s, 1.0)

        qT_s = qkv.tile([128, HPC * T], bf16)
        kT_s = qkv.tile([128, T], bf16)
        v_s = qkv.tile([128, NT * HD], bf16)   # natural [t%128, hd] per t-block
        aT_s = [qkv.tile([128, T], bf16, name=f"aT{h}") for h in range(HPC)]

        # weight/x loads: k first (first PE work), then the rest in need-order
        wk_s = wpool.tile([128, ND, HD], bf16)
        nc.gpsimd.dma_start(wk_s, wkT.rearrange("(d p) n -> p d n", p=128))
        cos_s = const.tile([128, T], bf16)
        sin_s = const.tile([128, T], bf16)

        xpool = ctx.enter_context(tc.tile_pool(name="xp", bufs=18))

        def load_x_quarter(tq):
            xts = []
            for d in range(ND):
                xt = xpool.tile([128, 512], bf16, tag="xt", name=f"xt{tq}_{d}")
                eng = nc.sync if d % 2 == 0 else nc.gpsimd
                eng.dma_start(
                    xt, xT[d * 128:(d + 1) * 128, tq * 512:(tq + 1) * 512])
                xts.append(xt)
            return xts

        # DMA issue order = need order: wk, x0, wq, x1, wv, cos/sin, x2,
        # wo, x3 (weights for later stages must not delay early x quarters)
        xq = [load_x_quarter(0)]
        wq_s = wpool.tile([128, ND, HPC * HD], bf16)
        nc.gpsimd.dma_start(wq_s, wqT.rearrange("(d p) n -> p d n", p=128))
        xq.append(load_x_quarter(1))
        wv_s = wpool.tile([128, ND, HD], bf16)
        nc.gpsimd.dma_start(wv_s, wvT.rearrange("(d p) n -> p d n", p=128))
        nc.gpsimd.dma_start(cos_s, cosT)
        nc.gpsimd.dma_start(sin_s, sinT)
        wo_s = wpool.tile([128, HPC, DIM], bf16)

        rp = ctx.enter_context(tc.tile_pool(name="rope", bufs=4))
        pps = ctx.enter_context(tc.tile_pool(name="pps", bufs=2, space="PSUM"))
        sps = ctx.enter_context(tc.tile_pool(name="sps", bufs=3, space="PSUM"))
        otp = ctx.enter_context(tc.tile_pool(name="otp", bufs=1, space="PSUM"))
        wops = ctx.enter_context(tc.tile_pool(name="wops", bufs=2, space="PSUM"))
        ppool = ctx.enter_context(tc.tile_pool(name="pp", bufs=20))
        rcp = ctx.enter_context(tc.tile_pool(name="rcp", bufs=2))
        ostage = ctx.enter_context(tc.tile_pool(name="ost", bufs=6))

        def rope(u, c0, t0):
            us = u[:, c0:c0 + 512]
            rot = rp.tile([128, 512], bf16, tag="rot")
            nc.sync.dma_start(rot[0:64, :], us[64:128, :])
            nc.sync.dma_start(rot[64:128, :], us[0:64, :])
            tmp = rp.tile([128, 512], bf16, tag="rtmp")
            nc.vector.tensor_mul(tmp, us, cos_s[:, t0:t0 + 512])
            nc.vector.tensor_mul(rot, rot, sin_s[:, t0:t0 + 512])
            nc.vector.tensor_add(us, tmp, rot)

        def proj(acc_tag, w_ap, xts, dst, c0):
            acc = pps.tile([128, 512], f32, tag="pps", name=acc_tag)
            for d in range(ND):
                nc.tensor.matmul(acc, w_ap(d), xts[d],
                                 start=(d == 0), stop=(d == ND - 1))
            nc.scalar.copy(dst[:, c0:c0 + 512], acc)

        def quarter_proj(tq, xts):
            t0 = tq * 512
            proj(f"k{tq}", lambda d: wk_s[:, d, :], xts, kT_s, t0)
            rope(kT_s, t0, t0)
            for h in range(HPC):
                proj(f"q{h}_{tq}",
                     lambda d, h=h: wq_s[:, d, h * HD:(h + 1) * HD],
                     xts, qT_s, h * T + t0)
                rope(qT_s, h * T + t0, t0)
            proj(f"v{tq}", lambda d: wv_s[:, d, :], xts, vT_stage, t0)
            for tb in range(tq * 4, tq * 4 + 4):
                vt = pps.tile([128, 128], bf16, tag="pps", name=f"vt{tb}")
                nc.tensor.transpose(
                    vt, vT_stage[:, tb * 128:(tb + 1) * 128], ident)
                nc.scalar.copy(v_s[:, tb * HD:(tb + 1) * HD], vt)

        vT_stage = qkv.tile([128, T], bf16)

        def attention(h, qc):
            qTh = qT_s[:, h * T + qc * 512:h * T + (qc + 1) * 512]
            nkb = 4 * qc + 4
            ptiles = []
            for kb in range(nkb):
                s_ps = sps.tile([128, 512], f32, tag="s", name=f"s{h}_{qc}_{kb}")
                nc.tensor.matmul(
                    s_ps, kT_s[:, kb * 128:(kb + 1) * 128], qTh,
                    start=True, stop=True)
                p_sb = ppool.tile([128, 512], bf16, tag="p",
                                  name=f"p{h}_{qc}_{kb}")
                nc.scalar.activation(p_sb, s_ps, Exp, scale=SCALE)
                if kb >= 4 * qc:
                    nc.gpsimd.affine_select(
                        out=p_sb, in_=p_sb,
                        compare_op=mybir.AluOpType.is_ge,
                        fill=0.0, base=qc * 512 - kb * 128,
                        channel_multiplier=-1, pattern=[[1, 512]])
                ptiles.append(p_sb)
            oT = otp.tile([128, 512], f32, tag="oT", name=f"oT{h}_{qc}")
            for kb in range(nkb):
                nc.tensor.matmul(
                    oT, v_s[:, kb * HD:(kb + 1) * HD], ptiles[kb],
                    start=(kb == 0), stop=(kb == nkb - 1))
            # dn = colsum(sum_kb P_kb): pairwise DVE tree then one ones-matmul
            lvl = list(ptiles)
            ti = 0
            while len(lvl) > 1:
                nxt = []
                for i in range(0, len(lvl) - 1, 2):
                    t = ppool.tile([128, 512], bf16, tag="pt",
                                   name=f"pt{h}_{qc}_{ti}")
                    ti += 1
                    nc.vector.tensor_add(t, lvl[i], lvl[i + 1])
                    nxt.append(t)
                if len(lvl) % 2:
                    nxt.append(lvl[-1])
                lvl = nxt
            dn = sps.tile([128, 512], f32, tag="s", name=f"dn{h}_{qc}")
            nc.tensor.matmul(dn, ones_s, lvl[0], start=True, stop=True)
            rec = rcp.tile([128, 512], f32, tag="rec")
            nc.vector.reciprocal_approx_fast(rec, dn)
            nc.vector.tensor_mul(
                aT_s[h][:, qc * 512:(qc + 1) * 512], oT, rec)

        def wo_block(qc):
            for tb in range(qc * 4, qc * 4 + 4):
                for n4 in range(4):
                    op = wops.tile([128, 512], f32, tag="op")
                    for h in range(HPC):
                        nc.tensor.matmul(
                            op, aT_s[h][:, tb * 128:(tb + 1) * 128],
                            wo_s[:, h, n4 * 512:(n4 + 1) * 512],
                            start=(h == 0), stop=(h == HPC - 1))
                    ob = ostage.tile([128, 512], bf16, tag="ob")
                    nc.vector.tensor_copy(ob, op)
                    eng = nc.sync if tb % 2 == 0 else nc.scalar
                    eng.dma_start(
                        out[tb * 128:(tb + 1) * 128,
                            n4 * 512:(n4 + 1) * 512], ob)

        for tq in range(NQC):
            if tq + 2 <= 3:
                xq.append(load_x_quarter(tq + 2))
            if tq == 0:
                nc.gpsimd.dma_start(
                    wo_s, woT.rearrange("(h p) n -> p h n", p=128))
            quarter_proj(tq, xq[tq])
            for h in range(HPC):
                attention(h, tq)
            wo_block(tq)

    nc.compile()
    return nc


def _shard_inputs(x, wq, wk, wv, wo, cos, sin):
    import ml_dtypes

    bf = ml_dtypes.bfloat16
    xTh = np.ascontiguousarray(x.reshape(T, DIM).T).astype(bf)
    cosTh = np.ascontiguousarray(cos.T).astype(bf)
    # rotate_half sign fold: out = u*cos + u_rot*sin_signed
    sinTh = np.ascontiguousarray(sin.T).copy()
    sinTh[: HD // 2, :] *= -1.0
    sinTh = sinTh.astype(bf)
    in_maps = []
    for c in range(NCORES):
        g = c // 2
        in_maps.append({
            "xT": xTh,
            "wqT": np.ascontiguousarray(
                wq[c * HPC * HD:(c + 1) * HPC * HD, :].T).astype(bf),
            "wkT": np.ascontiguousarray(wk[g * HD:(g + 1) * HD, :].T).astype(bf),
            "wvT": np.ascontiguousarray(wv[g * HD:(g + 1) * HD, :].T).astype(bf),
            "woT": np.ascontiguousarray(
                wo[:, c * HPC * HD:(c + 1) * HPC * HD].T).astype(bf),
            "cosT": cosTh,
            "sinT": sinTh,
        })
    return in_maps


def _get_exec():
    """Build (once) a cached jitted SPMD executable over the 8 cores.

    Mirrors bass2jax.run_bass_via_pjrt's multi-core branch, but caches the
    jitted callable so repeat kernel() calls don't re-trace/re-lower.
    """
    if "exec" in _CACHE:
        return _CACHE["exec"]

    import jax
    from jax.sharding import Mesh, PartitionSpec
    from jax.experimental.shard_map import shard_map
    from concourse import bass2jax
    import concourse.mybir as mybir

    if "nc" not in _CACHE:
        _CACHE["nc"] = _build_nc()
    nc = _CACHE["nc"]

    bass2jax.install_neuronx_cc_hook()

    part_name = (nc.partition_id_tensor.name
                 if nc.partition_id_tensor else None)
    in_names, out_names, out_avals = [], [], []
    for alloc in nc.m.functions[0].allocations:
        if not isinstance(alloc, mybir.MemoryLocationSet):
            continue
        name = alloc.memorylocations[0].name
        if alloc.kind == "ExternalInput":
            if name != part_name:
                in_names.append(name)
        elif alloc.kind == "ExternalOutput":
            out_names.append(name)
            out_avals.append(jax.core.ShapedArray(
                tuple(alloc.tensor_shape), mybir.dt.np(alloc.dtype)))

    bind_names = in_names + out_names
    if part_name is not None:
        bind_names = bind_names + [part_name]

    def _body(*args):
        operands = list(args)
        if part_name is not None:
            operands.append(bass2jax.partition_id_tensor())
        outs = bass2jax._bass_exec_p.bind(
            *operands,
            out_avals=tuple(out_avals),
            in_names=tuple(bind_names),
            out_names=tuple(out_names),
            lowering_input_output_aliases=(),
            sim_require_finite=True,
            sim_require_nnan=True,
            nc=nc,
        )
        return tuple(outs)

    devices = jax.devices()[:NCORES]
    mesh = Mesh(np.asarray(devices), ("core",))
    n_in = len(in_names)
    n_out = len(out_names)
    sharded = jax.jit(
        shard_map(
            _body, mesh=mesh,
            in_specs=(PartitionSpec("core"),) * (n_in + n_out),
            out_specs=(PartitionSpec("core"),) * n_out,
            check_rep=False,
        ),
        donate_argnums=tuple(range(n_in, n_in + n_out)),
        keep_unused=True,
    )
    _CACHE["body"] = _body
    _CACHE["exec"] = (sharded, in_names, out_names, out_avals, mesh)
    return _CACHE["exec"]


def _concat_inputs(in_maps, in_names):
    return [
        np.concatenate([in_maps[c][name] for c in range(NCORES)], axis=0)
        for name in in_names
    ]


def _zero_outs(out_avals):
    return [
        np.zeros((NCORES * a.shape[0], *a.shape[1:]), a.dtype)
        for a in out_avals
    ]


def kernel(**inputs):
    sharded, in_names, out_names, out_avals, _ = _get_exec()

    in_maps = _shard_inputs(
        np.asarray(inputs["x"], dtype=np.float32),
        np.asarray(inputs["wq"], dtype=np.float32),
        np.asarray(inputs["wk"], dtype=np.float32),
        np.asarray(inputs["wv"], dtype=np.float32),
        np.asarray(inputs["wo"], dtype=np.float32),
        np.asarray(inputs["cos"], dtype=np.float32),
        np.asarray(inputs["sin"], dtype=np.float32),
    )
    concat_in = _concat_inputs(in_maps, in_names)
    out_arrs = sharded(*concat_in, *_zero_outs(out_avals))

    full = np.asarray(out_arrs[out_names.index("out")])
    acc = full.reshape(NCORES, T, DIM).astype(np.float32).sum(axis=0)
    return acc.reshape(1, T, DIM)
